# revision 9
# baseline (speedup 1.0000x reference)
"""ARMA filter on 8 NeuronCores via Bass/Tile — v2 (hardware-loop AR scan).

y_t = tanh(ma_t + tanh(arnet(y_{t-8..t-1}))), ma = tanh(causal 9-tap conv(x)).

Sharding: data-parallel over batch (64 rows -> 8 cores x 8 rows). Identical
SPMD program on every core; no collectives.

v2 restructures v1 for end-to-end wall clock, keeping the same math:
  - The AR scan is a tc.For_i hardware loop (T/8 iterations x ~135-instr
    body with register-offset SBUF addressing) instead of a fully unrolled
    ~33k-instruction program. Python build time, Tile scheduling, walrus
    compile time, and NEFF size all shrink ~20x.
  - x ships unpadded ([128, 2*BL*T] bf16); the 8-frame causal pad is
    materialized in SBUF (dynamic DRAM addressing isn't supported by this
    lowering, so all DMAs are static: per-time-tile x loads, one final
    y store).
  - Custom PJRT runner: outputs are not passed as donated zero inputs
    (the kernel writes every yout element), saving a 67MB transfer over
    the ~70MB/s axon tunnel.
  - Program build + jit AOT compile happen once per process and are
    triggered at module import.

Per-core layout ("transposed"): feature dim on partitions. Per-step tensors
are [128 partitions (f half), 16 free (f-chunk m in {0,1} x batch b in 0..8)].
All matmul data is bf16 (fp32 PSUM accumulation); tolerance is 2e-2.
"""

import os

os.environ.setdefault("JAX_PLATFORMS", "axon,cpu")

import numpy as np
import ml_dtypes

F = 256
MASZ = 8  # MA taps minus current frame (conv length 9)
ARSZ = 8
B_TOT = 64
NCORES = 8
BL = B_TOT // NCORES  # 8 batch rows per core
T_FULL = 2048
WPACK_COLS = 4096 + 4608 + 256 + 2  # wtap | wma | barw | bma

_CACHE = {}


def _bf16(a):
    return np.asarray(a, dtype=np.float32).astype(ml_dtypes.bfloat16)


def _patch_tile_drain():
    """Split the Tile tail-drain's sem waits across single-wait NOPs.

    The nix walrus in this container rejects instructions with >2 sync
    waits ("Too many sync wait commands"), and TileContext's exit drain
    waits on the whole vector clock.
    """
    import concourse.tile as tile
    import concourse.mybir as mybir
    from concourse.vector_clock import ScopedClock

    if getattr(tile.TileContext, "_arma_drain_patched", False):
        return

    def _drain_and_barrier(self, tick_clock, wait_clock):
        nc = self.nc
        holder = nc.sync.nop(nofuse=True)
        wait_clock.add_sem_waits(
            holder.ins, ScopedClock({None: tick_clock.global_clock})
        )
        si = holder.ins.sync_info
        waits = list(si.on_wait) if si else []
        if si:
            si.on_wait[:] = waits[:1]
        for w in waits[1:]:
            n = nc.sync.nop(nofuse=True)
            if n.ins.sync_info is None:
                n.ins.sync_info = mybir.SyncInfo(on_wait=[w], on_update=[])
            else:
                n.ins.sync_info.on_wait.append(w)
        nc.sync.drain()
        nc.all_engine_barrier()
        assert self.sems is not None
        popped = nc._tile_sem_poison_stack.pop()
        assert popped is self._sem_poison
        nc.clear_and_free_semaphores(list(self.sems.allocated().values()))
        nc.all_engine_barrier()

    tile.TileContext._drain_and_barrier = _drain_and_barrier
    tile.TileContext._arma_drain_patched = True


def _split_waits(nc):
    """Cap every instruction at one sync wait.

    This container's walrus rejects instructions with more than one or two
    sync waits ("Too many sync wait commands"). Hoist extra waits onto
    same-engine NOPs inserted immediately before the instruction.
    """
    import concourse.mybir as mybir

    eng = {
        mybir.EngineType.Activation: nc.scalar,
        mybir.EngineType.DVE: nc.vector,
        mybir.EngineType.PE: nc.tensor,
        mybir.EngineType.Pool: nc.gpsimd,
        mybir.EngineType.SP: nc.sync,
    }
    f = nc.m.functions[0]

    def steal_nop(engine):
        nop = eng[engine].nop(nofuse=True).ins
        for bb in f.blocks:
            l = bb.instructions
            if l and l[-1].name == nop.name:
                l.pop()
                break
        return nop

    for b in f.blocks:
        il = b.instructions
        i = 0
        while i < len(il):
            inst = il[i]
            si = inst.sync_info
            if si is not None and len(si.on_wait) > 1:
                extras = list(si.on_wait[1:])
                si.on_wait[:] = si.on_wait[:1]
                for w in extras:
                    nop = steal_nop(inst.engine)
                    nop.sync_info = mybir.SyncInfo(on_wait=[w], on_update=[])
                    il.insert(i, nop)
                    i += 1
            i += 1


def _build_program(T):
    import concourse.bass as bass
    import concourse.mybir as mybir
    import concourse.tile as tile
    from concourse.bass import ds

    _patch_tile_drain()
    dt = mybir.dt
    TT = min(512, T)
    nc = bass.Bass("TRN2", target_bir_lowering=False, debug=False,
                   num_devices=NCORES)

    xt = nc.declare_dram_parameter("xt", [BL * T, 256], dt.bfloat16,
                                   isOutput=False)
    # all weights in one tensor: fewer, larger tunnel transfers
    # cols: wtap 4096 | wma 4608 | barw 256 (partition 0) | bma 2
    wpack = nc.declare_dram_parameter("wpack", [128, WPACK_COLS],
                                      dt.bfloat16, isOutput=False)
    # y ships as two u8 byte planes (hi | lo): the hi (sign+exponent) plane
    # is low-entropy and the tunnel compresses it, cutting D2H wire time.
    yout = nc.declare_dram_parameter("yout", [128, T * 32], dt.uint8,
                                     isOutput=True)

    Tanh = mybir.ActivationFunctionType.Tanh
    nblk = T // ARSZ

    with tile.TileContext(nc) as tc:
        with (
            tc.tile_pool(name="w", bufs=1) as wpool,
            tc.tile_pool(name="ybuf", bufs=1) as ypool,
            tc.tile_pool(name="mabuf", bufs=1) as mapool,
            tc.tile_pool(name="xs", bufs=2) as xpool,
            tc.tile_pool(name="sc", bufs=1) as scpool,
            tc.tile_pool(name="pma", bufs=2, space="PSUM") as pmapool,
            tc.tile_pool(name="psc", bufs=1, space="PSUM") as pscpool,
        ):
            wpack_sb = wpool.tile([128, WPACK_COLS], dt.bfloat16)
            nc.sync.dma_start(out=wpack_sb[:], in_=wpack[:])
            wtap_sb = wpack_sb[:, 0:4096]
            wma_sb = wpack_sb[:, 4096:8704]
            barw_sb = wpack_sb[0:1, 8704:8960]
            bma_sb = wpack_sb[:, 8960:8962]
            ones_sb = wpool.tile([1, 64], dt.bfloat16)
            nc.vector.memset(ones_sb[:], 1.0)

            # y staging for the final output DMA: col t*16 + m*8 + b
            y_st = ypool.tile([128, T * 16], dt.bfloat16)

            ma_sb = mapool.tile([128, T * 16], dt.bfloat16, name="ma_sb",
                                tag="ma_sb")
            ma3 = ma_sb[:].rearrange("p (t c) -> p t c", c=16)

            def w_tile(d, k, m):
                i = ((d - 1) * 4 + k * 2 + m) * 128
                return wtap_sb[:, i:i + 128]

            # ---------------- MA phase (static) ----------------
            # x ships in natural [BL*T, 256] layout; the feature-to-partition
            # transpose happens on-device via xbar transpose DMAs. Window is
            # 16-row aligned (HALO=16 head cols; taps read cols c+w+8).
            n_tt = T // TT
            HALO = 16
            TTH = TT + HALO
            for tt in range(n_tt):
                xt_t = xpool.tile([128, 16 * TTH], dt.bfloat16,
                                  name="xt_t", tag="xt_t")
                x3 = xt_t[:].rearrange("p (s c) -> p s c", s=16)
                for b in range(BL):
                    for kf in range(2):
                        s = kf * BL + b
                        if tt == 0:
                            nc.vector.memset(x3[:, s:s + 1, 0:HALO], 0.0)
                            nc.sync.dma_start(
                                out=x3[:, s, HALO:TTH],
                                in_=xt[b * T:b * T + TT,
                                       kf * 128:(kf + 1) * 128],
                                transpose=True)
                        else:
                            r0 = b * T + tt * TT - HALO
                            nc.sync.dma_start(
                                out=x3[:, s, 0:TTH],
                                in_=xt[r0:r0 + TTH,
                                       kf * 128:(kf + 1) * 128],
                                transpose=True)
                for b in range(BL):
                    for m in range(2):
                        ps = pmapool.tile([128, TT], dt.float32)
                        for kk in range(18):
                            w = kk // 2
                            kf = kk % 2
                            c0 = (kf * BL + b) * TTH + w + (HALO - MASZ)
                            nc.tensor.matmul(
                                ps[:],
                                lhsT=wma_sb[:, (kk * 2 + m) * 128:
                                            (kk * 2 + m + 1) * 128],
                                rhs=xt_t[:, c0:c0 + TT],
                                start=(kk == 0),
                                stop=(kk == 17),
                            )
                        dst = ma3[:, tt * TT:(tt + 1) * TT,
                                  m * 8 + b:m * 8 + b + 1]
                        nc.scalar.activation(
                            dst, ps[:].rearrange("p (t o) -> p t o", o=1),
                            Tanh, bias=bma_sb[:, m:m + 1])

            # ---------------- AR scan (hardware loop) ----------------
            # 16 steps (2 psum blocks) per iteration. All matmul/ACT APs are
            # static: y history lives in a 24-slot rolling window (slot j+8
            # holds step j of this body; slots 0..7 are the previous body's
            # tail, refreshed by one static slide copy). Only the per-body
            # ma stage-in and y write-back use dynamic (register-offset)
            # addressing — dynamic APs burn engine index registers at
            # lowering, so keep them rare.
            ywin = ypool.tile([128, 24 * 16], dt.bfloat16)
            nc.vector.memset(ywin[:], 0.0)
            yw3 = ywin[:].rearrange("p (s c) -> p s c", c=16)
            mawin = ypool.tile([128, 256], dt.bfloat16)
            psb = [pscpool.tile([128, 128], dt.float32, name=f"psc{h}",
                                tag=f"psc{h}") for h in range(2)]
            ar_t = scpool.tile([128, 16], dt.bfloat16, name="ar", tag="ar")
            z_t = scpool.tile([128, 16], dt.bfloat16, name="z", tag="z")

            with tc.For_i(0, T // 16, 1) as b2:
                c256 = b2 * 256
                # previous body's last 8 steps -> history slots 0..7
                nc.vector.tensor_copy(ywin[:, 0:128], ywin[:, 16 * 16:24 * 16])
                nc.vector.tensor_copy(mawin[:], ma_sb[:, ds(c256, 256)])
                for half in range(2):
                    ps = psb[half]
                    ps3 = ps[:].rearrange("p (i c) -> p i c", c=16)

                    def fire(d, i0, n, stop=False):
                        # tap lag d applied to body steps j0..j0+n-1
                        j0 = half * 8 + i0
                        for m in range(2):
                            for k in range(2):
                                nc.tensor.matmul(
                                    ps3[:, i0:i0 + n, m * 8:m * 8 + 8],
                                    lhsT=w_tile(d, k, m),
                                    rhs=yw3[:, j0 - d + 8:j0 - d + 8 + n,
                                            k * 8:k * 8 + 8],
                                    start=False, stop=stop and k == 1,
                                    skip_group_check=True,
                                )

                    # init with b_ar (K=1 matmul, start=True covers all cols)
                    for m in range(2):
                        nc.tensor.matmul(
                            ps3[:, :, m * 8:m * 8 + 8],
                            lhsT=barw_sb[0:1, m * 128:(m + 1) * 128],
                            rhs=ones_sb[0:1, :], start=True, stop=False,
                            skip_group_check=True,
                        )
                    for i in range(ARSZ):
                        j = half * 8 + i
                        if i == 0:
                            fire(8, 0, 8)
                        if i in (0, 4):
                            for d in (4, 5, 6, 7):
                                fire(d, i, 4)
                        if i % 2 == 0:
                            for d in (2, 3):
                                fire(d, i, 2)
                        fire(1, i, 1, stop=True)
                        nc.scalar.activation(
                            ar_t[:], ps[:, i * 16:(i + 1) * 16], Tanh)
                        nc.vector.tensor_add(
                            z_t[:], ar_t[:], mawin[:, j * 16:(j + 1) * 16])
                        nc.scalar.activation(
                            ywin[:, (j + 8) * 16:(j + 9) * 16], z_t[:], Tanh)
                nc.vector.tensor_copy(y_st[:, ds(c256, 256)],
                                      ywin[:, 128:24 * 16])

            # split y_st bytes into planes; reuse ma_sb's slot (ma is dead
            # after the loop; Tile's WAR deps order the writes after the
            # last ma read)
            yplanes = mapool.tile([128, T * 32], dt.uint8, name="yplanes",
                                  tag="ma_sb")
            yu8 = y_st[:].bitcast(dt.uint8).rearrange(
                "p (t two) -> p t two", two=2)
            nc.vector.tensor_copy(
                yplanes[:, 0:T * 16].rearrange("p (t o) -> p t o", o=1),
                yu8[:, :, 1:2])
            nc.vector.tensor_copy(
                yplanes[:, T * 16:T * 32].rearrange("p (t o) -> p t o", o=1),
                yu8[:, :, 0:1])
            nc.sync.dma_start(out=yout[:], in_=yplanes[:])
    _split_waits(nc)
    return nc


def _make_runner(nc):
    import jax
    from jax.sharding import Mesh, NamedSharding, PartitionSpec
    from jax.experimental.shard_map import shard_map
    import concourse.mybir as mybir
    from concourse.bass2jax import (
        install_neuronx_cc_hook, partition_id_tensor, _bass_exec_p)

    install_neuronx_cc_hook()
    partition_name = (nc.partition_id_tensor.name
                      if nc.partition_id_tensor else None)
    in_names = []
    out_names = []
    out_avals = []
    in_shapes = []
    for alloc in nc.m.functions[0].allocations:
        if not isinstance(alloc, mybir.MemoryLocationSet):
            continue
        name = alloc.memorylocations[0].name
        if alloc.kind == "ExternalInput":
            if name != partition_name:
                in_names.append(name)
                in_shapes.append((tuple(alloc.tensor_shape),
                                  mybir.dt.np(alloc.dtype)))
        elif alloc.kind == "ExternalOutput":
            out_names.append(name)
            out_avals.append(jax.core.ShapedArray(
                tuple(alloc.tensor_shape), mybir.dt.np(alloc.dtype)))
    n_params = len(in_names)
    names_all = list(in_names)
    if partition_name is not None:
        names_all.append(partition_name)

    def _body(*args):
        operands = list(args)
        if partition_name is not None:
            operands.append(partition_id_tensor())
        outs = _bass_exec_p.bind(
            *operands,
            out_avals=tuple(out_avals),
            in_names=tuple(names_all),
            out_names=tuple(out_names),
            lowering_input_output_aliases=(),
            sim_require_finite=True,
            sim_require_nnan=True,
            nc=nc,
        )
        return tuple(outs)

    devices = jax.devices()[:NCORES]
    assert len(devices) == NCORES
    mesh = Mesh(np.asarray(devices), ("core",))
    sharding = NamedSharding(mesh, PartitionSpec("core"))
    jitted = jax.jit(
        shard_map(_body, mesh=mesh,
                  in_specs=(PartitionSpec("core"),) * n_params,
                  out_specs=(PartitionSpec("core"),) * len(out_names),
                  check_rep=False),
        keep_unused=True)
    global_shapes = [jax.ShapeDtypeStruct((NCORES * s[0], *s[1:]), d)
                     for s, d in in_shapes]
    compiled = jitted.lower(*global_shapes).compile()
    return compiled, in_names, out_names, devices, sharding


def _get_state(T):
    if T not in _CACHE:
        nc = _build_program(T)
        _CACHE[T] = (nc, _make_runner(nc))
    return _CACHE[T]


def _prep_weights(W_ar, b_ar, W_ma, b_ma):
    # wtap: tap lag d in 1..8, tile (k, m):
    #   wtap[p, ((d-1)*4 + k*2 + m)*128 + q] = W_ar[m*128+q, (8-d)*256 + k*128 + p]
    wt = np.empty((128, ARSZ * 4 * 128), np.float32)
    for d in range(1, 9):
        blkc = W_ar[:, (8 - d) * F:(9 - d) * F]  # [fout, fin]
        for k in range(2):
            for m in range(2):
                tilev = blkc[m * 128:(m + 1) * 128, k * 128:(k + 1) * 128].T
                i = ((d - 1) * 4 + k * 2 + m) * 128
                wt[:, i:i + 128] = tilev
    # wma: tile (kk, m): wma[p, (kk*2+m)*128+q] = W_ma[m*128+q, kk*128+p]
    wm = np.empty((128, 18 * 2 * 128), np.float32)
    WmaT = W_ma.T  # [2304, 256]
    for kk in range(18):
        for m in range(2):
            i = (kk * 2 + m) * 128
            wm[:, i:i + 128] = WmaT[kk * 128:(kk + 1) * 128,
                                    m * 128:(m + 1) * 128]
    pack = np.zeros((128, WPACK_COLS), np.float32)
    pack[:, 0:4096] = wt
    pack[:, 4096:8704] = wm
    pack[0, 8704:8960] = b_ar
    pack[:, 8960] = b_ma[:128]
    pack[:, 8961] = b_ma[128:]
    return _bf16(pack)


def _unshard_core(yc, T):
    # y[b, t, m*128+p] = yc[p, t*16+m*8+b]  for one core
    a = yc.reshape(128, T, 2, BL).transpose(3, 1, 2, 0)
    return a.astype(np.float32).reshape(BL, T, F)


def _unplane_core(pc, T):
    # pc: [128, T*32] u8 = hi plane | lo plane -> bf16 [128, T*16]
    u16 = (pc[:, :T * 16].astype(np.uint16) << 8) | pc[:, T * 16:]
    return u16.view(ml_dtypes.bfloat16)


def kernel(x, W_ar, b_ar, W_ma, b_ma):
    import jax

    T = int(os.environ.get("ARMA_T", T_FULL))
    x = np.asarray(x, np.float32)[:, :T, :]
    W_ar = np.asarray(W_ar, np.float32)
    b_ar = np.asarray(b_ar, np.float32)
    W_ma = np.asarray(W_ma, np.float32)
    b_ma = np.asarray(b_ma, np.float32)

    nc, (compiled, in_names, out_names, devices, sharding) = _get_state(T)

    wdict = {"wpack": _prep_weights(W_ar, b_ar, W_ma, b_ma)}

    if int(os.environ.get("ARMA_TRACE", "0")):
        from concourse.bass_utils import run_bass_kernel_spmd
        in_maps = []
        for c in range(NCORES):
            m = dict(wdict)
            m["xt"] = x[c * BL:(c + 1) * BL].reshape(BL * T, F).astype(
                ml_dtypes.bfloat16)
            in_maps.append({n: m[n] for n in in_names})
        try:
            res = run_bass_kernel_spmd(nc, in_maps, list(range(NCORES)),
                                       trace=True)
        except ModuleNotFoundError:
            # NTFF profiling hooks unavailable in this container
            res = run_bass_kernel_spmd(nc, in_maps, list(range(NCORES)))
        kernel.last_results = res
        out = np.empty((B_TOT, T, F), np.float32)
        for c in range(NCORES):
            out[c * BL:(c + 1) * BL] = _unshard_core(
                _unplane_core(np.asarray(res.results[c]["yout"]), T), T)
        return out

    kernel.last_results = None
    import time as _time
    _dbg = int(os.environ.get("ARMA_DEBUG", "0"))
    _t = _time.perf_counter

    def _mark(label, t0):
        if _dbg:
            print(f"[arma] {label}: {_t() - t0:.2f}s", flush=True)
        return _t()

    t0 = _t()
    # Per-core shards go to their device as soon as they're prepared, so
    # host-side bf16 casts overlap the (~40MB/s for real data) tunnel
    # transfers.
    x_shards = []
    for c in range(NCORES):
        xc = x[c * BL:(c + 1) * BL].reshape(BL * T, F).astype(
            ml_dtypes.bfloat16)
        x_shards.append(jax.device_put(xc, devices[c]))
    w_shards = {n: [jax.device_put(wdict[n], d) for d in devices]
                for n in in_names if n != "xt"}
    t0 = _mark("puts dispatched", t0)

    def garr(name, shards):
        shp = shards[0].shape
        return jax.make_array_from_single_device_arrays(
            (NCORES * shp[0], *shp[1:]), sharding, shards)

    args = []
    for n in in_names:
        args.append(garr(n, x_shards if n == "xt" else w_shards[n]))
    t0 = _mark("garr", t0)
    outs = compiled(*args)
    yg = outs[0]
    jax.block_until_ready(yg)
    t0 = _mark("exec+transfers", t0)
    out = np.empty((B_TOT, T, F), np.float32)
    shard_map_ = {s.index[0].start or 0: s.data for s in yg.addressable_shards}
    datas = [shard_map_[c * 128] for c in range(NCORES)]
    for d in datas:  # start all D2H copies before converting any
        try:
            d.copy_to_host_async()
        except Exception:
            pass
    for c in range(NCORES):
        out[c * BL:(c + 1) * BL] = _unshard_core(
            _unplane_core(np.asarray(datas[c]), T), T)
    _mark("fetch+unshard", t0)
    return out


def _enable_jax_cache():
    try:
        import jax
        jax.config.update("jax_compilation_cache_dir",
                          "/var/tmp/arma_jax_cache")
        jax.config.update("jax_persistent_cache_min_compile_time_secs", 0.0)
        jax.config.update("jax_persistent_cache_min_entry_size_bytes", -1)
    except Exception:
        pass


# One-time setup (imports, program build, AOT compile) at module import.
if not int(os.environ.get("ARMA_NO_PREBUILD", "0")):
    try:
        _enable_jax_cache()
        _get_state(int(os.environ.get("ARMA_T", T_FULL)))
    except Exception:
        _CACHE.clear()


# revision 10
# speedup vs baseline: 1.0366x; 1.0366x over previous
"""ARMA filter on 8 NeuronCores via Bass/Tile — v2 (hardware-loop AR scan).

y_t = tanh(ma_t + tanh(arnet(y_{t-8..t-1}))), ma = tanh(causal 9-tap conv(x)).

Sharding: data-parallel over batch (64 rows -> 8 cores x 8 rows). Identical
SPMD program on every core; no collectives.

v2 restructures v1 for end-to-end wall clock, keeping the same math:
  - The AR scan is a tc.For_i hardware loop (T/8 iterations x ~135-instr
    body with register-offset SBUF addressing) instead of a fully unrolled
    ~33k-instruction program. Python build time, Tile scheduling, walrus
    compile time, and NEFF size all shrink ~20x.
  - x ships unpadded ([128, 2*BL*T] bf16); the 8-frame causal pad is
    materialized in SBUF (dynamic DRAM addressing isn't supported by this
    lowering, so all DMAs are static: per-time-tile x loads, one final
    y store).
  - Custom PJRT runner: outputs are not passed as donated zero inputs
    (the kernel writes every yout element), saving a 67MB transfer over
    the ~70MB/s axon tunnel.
  - Program build + jit AOT compile happen once per process and are
    triggered at module import.

Per-core layout ("transposed"): feature dim on partitions. Per-step tensors
are [128 partitions (f half), 16 free (f-chunk m in {0,1} x batch b in 0..8)].
All matmul data is bf16 (fp32 PSUM accumulation); tolerance is 2e-2.
"""

import os

os.environ.setdefault("JAX_PLATFORMS", "axon,cpu")

import numpy as np
import ml_dtypes

F = 256
MASZ = 8  # MA taps minus current frame (conv length 9)
ARSZ = 8
B_TOT = 64
NCORES = 8
BL = B_TOT // NCORES  # 8 batch rows per core
T_FULL = 2048
WPACK_COLS = 4096 + 4608 + 256 + 2  # wtap | wma | barw | bma

_CACHE = {}


def _bf16(a):
    return np.asarray(a, dtype=np.float32).astype(ml_dtypes.bfloat16)


def _patch_tile_drain():
    """Split the Tile tail-drain's sem waits across single-wait NOPs.

    The nix walrus in this container rejects instructions with >2 sync
    waits ("Too many sync wait commands"), and TileContext's exit drain
    waits on the whole vector clock.
    """
    import concourse.tile as tile
    import concourse.mybir as mybir
    from concourse.vector_clock import ScopedClock

    if getattr(tile.TileContext, "_arma_drain_patched", False):
        return

    def _drain_and_barrier(self, tick_clock, wait_clock):
        nc = self.nc
        holder = nc.sync.nop(nofuse=True)
        wait_clock.add_sem_waits(
            holder.ins, ScopedClock({None: tick_clock.global_clock})
        )
        si = holder.ins.sync_info
        waits = list(si.on_wait) if si else []
        if si:
            si.on_wait[:] = waits[:1]
        for w in waits[1:]:
            n = nc.sync.nop(nofuse=True)
            if n.ins.sync_info is None:
                n.ins.sync_info = mybir.SyncInfo(on_wait=[w], on_update=[])
            else:
                n.ins.sync_info.on_wait.append(w)
        nc.sync.drain()
        nc.all_engine_barrier()
        assert self.sems is not None
        popped = nc._tile_sem_poison_stack.pop()
        assert popped is self._sem_poison
        nc.clear_and_free_semaphores(list(self.sems.allocated().values()))
        nc.all_engine_barrier()

    tile.TileContext._drain_and_barrier = _drain_and_barrier
    tile.TileContext._arma_drain_patched = True


def _split_waits(nc):
    """Cap every instruction at one sync wait.

    This container's walrus rejects instructions with more than one or two
    sync waits ("Too many sync wait commands"). Hoist extra waits onto
    same-engine NOPs inserted immediately before the instruction.
    """
    import concourse.mybir as mybir

    eng = {
        mybir.EngineType.Activation: nc.scalar,
        mybir.EngineType.DVE: nc.vector,
        mybir.EngineType.PE: nc.tensor,
        mybir.EngineType.Pool: nc.gpsimd,
        mybir.EngineType.SP: nc.sync,
    }
    f = nc.m.functions[0]

    def steal_nop(engine):
        nop = eng[engine].nop(nofuse=True).ins
        for bb in f.blocks:
            l = bb.instructions
            if l and l[-1].name == nop.name:
                l.pop()
                break
        return nop

    for b in f.blocks:
        il = b.instructions
        i = 0
        while i < len(il):
            inst = il[i]
            si = inst.sync_info
            if si is not None and len(si.on_wait) > 1:
                extras = list(si.on_wait[1:])
                si.on_wait[:] = si.on_wait[:1]
                for w in extras:
                    nop = steal_nop(inst.engine)
                    nop.sync_info = mybir.SyncInfo(on_wait=[w], on_update=[])
                    il.insert(i, nop)
                    i += 1
            i += 1


def _build_program(T):
    import concourse.bass as bass
    import concourse.mybir as mybir
    import concourse.tile as tile
    from concourse.bass import ds

    _patch_tile_drain()
    dt = mybir.dt
    TT = min(512, T)
    nc = bass.Bass("TRN2", target_bir_lowering=False, debug=False,
                   num_devices=NCORES)

    xt = nc.declare_dram_parameter("xt", [BL * T, 256], dt.bfloat16,
                                   isOutput=False)
    # all weights in one tensor: fewer, larger tunnel transfers
    # cols: wtap 4096 | wma 4608 | barw 256 (partition 0) | bma 2
    wpack = nc.declare_dram_parameter("wpack", [128, WPACK_COLS],
                                      dt.bfloat16, isOutput=False)
    yout = nc.declare_dram_parameter("yout", [128, T * 16], dt.bfloat16,
                                     isOutput=True)

    Tanh = mybir.ActivationFunctionType.Tanh
    nblk = T // ARSZ

    with tile.TileContext(nc) as tc:
        with (
            tc.tile_pool(name="w", bufs=1) as wpool,
            tc.tile_pool(name="ybuf", bufs=1) as ypool,
            tc.tile_pool(name="mabuf", bufs=1) as mapool,
            tc.tile_pool(name="xs", bufs=2) as xpool,
            tc.tile_pool(name="sc", bufs=1) as scpool,
            tc.tile_pool(name="pma", bufs=2, space="PSUM") as pmapool,
            tc.tile_pool(name="psc", bufs=1, space="PSUM") as pscpool,
        ):
            wpack_sb = wpool.tile([128, WPACK_COLS], dt.bfloat16)
            nc.sync.dma_start(out=wpack_sb[:], in_=wpack[:])
            wtap_sb = wpack_sb[:, 0:4096]
            wma_sb = wpack_sb[:, 4096:8704]
            barw_sb = wpack_sb[0:1, 8704:8960]
            bma_sb = wpack_sb[:, 8960:8962]
            ones_sb = wpool.tile([1, 64], dt.bfloat16)
            nc.vector.memset(ones_sb[:], 1.0)

            # y staging for the final output DMA: col t*16 + m*8 + b
            y_st = ypool.tile([128, T * 16], dt.bfloat16)

            ma_sb = mapool.tile([128, T * 16], dt.bfloat16, name="ma_sb",
                                tag="ma_sb")
            ma3 = ma_sb[:].rearrange("p (t c) -> p t c", c=16)

            def w_tile(d, k, m):
                i = ((d - 1) * 4 + k * 2 + m) * 128
                return wtap_sb[:, i:i + 128]

            # ---------------- MA phase (static) ----------------
            # x ships in natural [BL*T, 256] layout; the feature-to-partition
            # transpose happens on-device via xbar transpose DMAs. Window is
            # 16-row aligned (HALO=16 head cols; taps read cols c+w+8).
            n_tt = T // TT
            HALO = 16
            TTH = TT + HALO
            for tt in range(n_tt):
                xt_t = xpool.tile([128, 16 * TTH], dt.bfloat16,
                                  name="xt_t", tag="xt_t")
                x3 = xt_t[:].rearrange("p (s c) -> p s c", s=16)
                for b in range(BL):
                    for kf in range(2):
                        s = kf * BL + b
                        if tt == 0:
                            nc.vector.memset(x3[:, s:s + 1, 0:HALO], 0.0)
                            nc.sync.dma_start(
                                out=x3[:, s, HALO:TTH],
                                in_=xt[b * T:b * T + TT,
                                       kf * 128:(kf + 1) * 128],
                                transpose=True)
                        else:
                            r0 = b * T + tt * TT - HALO
                            nc.sync.dma_start(
                                out=x3[:, s, 0:TTH],
                                in_=xt[r0:r0 + TTH,
                                       kf * 128:(kf + 1) * 128],
                                transpose=True)
                for b in range(BL):
                    for m in range(2):
                        ps = pmapool.tile([128, TT], dt.float32)
                        for kk in range(18):
                            w = kk // 2
                            kf = kk % 2
                            c0 = (kf * BL + b) * TTH + w + (HALO - MASZ)
                            nc.tensor.matmul(
                                ps[:],
                                lhsT=wma_sb[:, (kk * 2 + m) * 128:
                                            (kk * 2 + m + 1) * 128],
                                rhs=xt_t[:, c0:c0 + TT],
                                start=(kk == 0),
                                stop=(kk == 17),
                            )
                        dst = ma3[:, tt * TT:(tt + 1) * TT,
                                  m * 8 + b:m * 8 + b + 1]
                        nc.scalar.activation(
                            dst, ps[:].rearrange("p (t o) -> p t o", o=1),
                            Tanh, bias=bma_sb[:, m:m + 1])

            # ---------------- AR scan (hardware loop) ----------------
            # 16 steps (2 psum blocks) per iteration. All matmul/ACT APs are
            # static: y history lives in a 24-slot rolling window (slot j+8
            # holds step j of this body; slots 0..7 are the previous body's
            # tail, refreshed by one static slide copy). Only the per-body
            # ma stage-in and y write-back use dynamic (register-offset)
            # addressing — dynamic APs burn engine index registers at
            # lowering, so keep them rare.
            ywin = ypool.tile([128, 24 * 16], dt.bfloat16)
            nc.vector.memset(ywin[:], 0.0)
            yw3 = ywin[:].rearrange("p (s c) -> p s c", c=16)
            mawin = ypool.tile([128, 256], dt.bfloat16)
            psb = [pscpool.tile([128, 128], dt.float32, name=f"psc{h}",
                                tag=f"psc{h}") for h in range(2)]
            ar_t = scpool.tile([128, 16], dt.bfloat16, name="ar", tag="ar")
            z_t = scpool.tile([128, 16], dt.bfloat16, name="z", tag="z")

            with tc.For_i(0, T // 16, 1) as b2:
                c256 = b2 * 256
                # previous body's last 8 steps -> history slots 0..7
                nc.vector.tensor_copy(ywin[:, 0:128], ywin[:, 16 * 16:24 * 16])
                nc.vector.tensor_copy(mawin[:], ma_sb[:, ds(c256, 256)])
                for half in range(2):
                    ps = psb[half]
                    ps3 = ps[:].rearrange("p (i c) -> p i c", c=16)

                    def fire(d, i0, n, stop=False):
                        # tap lag d applied to body steps j0..j0+n-1
                        j0 = half * 8 + i0
                        for m in range(2):
                            for k in range(2):
                                nc.tensor.matmul(
                                    ps3[:, i0:i0 + n, m * 8:m * 8 + 8],
                                    lhsT=w_tile(d, k, m),
                                    rhs=yw3[:, j0 - d + 8:j0 - d + 8 + n,
                                            k * 8:k * 8 + 8],
                                    start=False, stop=stop and k == 1,
                                    skip_group_check=True,
                                )

                    # init with b_ar (K=1 matmul, start=True covers all cols)
                    for m in range(2):
                        nc.tensor.matmul(
                            ps3[:, :, m * 8:m * 8 + 8],
                            lhsT=barw_sb[0:1, m * 128:(m + 1) * 128],
                            rhs=ones_sb[0:1, :], start=True, stop=False,
                            skip_group_check=True,
                        )
                    for i in range(ARSZ):
                        j = half * 8 + i
                        if i == 0:
                            fire(8, 0, 8)
                        if i in (0, 4):
                            for d in (4, 5, 6, 7):
                                fire(d, i, 4)
                        if i % 2 == 0:
                            for d in (2, 3):
                                fire(d, i, 2)
                        fire(1, i, 1, stop=True)
                        nc.scalar.activation(
                            ar_t[:], ps[:, i * 16:(i + 1) * 16], Tanh)
                        nc.vector.tensor_add(
                            z_t[:], ar_t[:], mawin[:, j * 16:(j + 1) * 16])
                        nc.scalar.activation(
                            ywin[:, (j + 8) * 16:(j + 9) * 16], z_t[:], Tanh)
                nc.vector.tensor_copy(y_st[:, ds(c256, 256)],
                                      ywin[:, 128:24 * 16])

            nc.sync.dma_start(out=yout[:], in_=y_st[:])
    _split_waits(nc)
    return nc


def _make_runner(nc):
    import jax
    from jax.sharding import Mesh, NamedSharding, PartitionSpec
    from jax.experimental.shard_map import shard_map
    import concourse.mybir as mybir
    from concourse.bass2jax import (
        install_neuronx_cc_hook, partition_id_tensor, _bass_exec_p)

    install_neuronx_cc_hook()
    partition_name = (nc.partition_id_tensor.name
                      if nc.partition_id_tensor else None)
    in_names = []
    out_names = []
    out_avals = []
    in_shapes = []
    for alloc in nc.m.functions[0].allocations:
        if not isinstance(alloc, mybir.MemoryLocationSet):
            continue
        name = alloc.memorylocations[0].name
        if alloc.kind == "ExternalInput":
            if name != partition_name:
                in_names.append(name)
                in_shapes.append((tuple(alloc.tensor_shape),
                                  mybir.dt.np(alloc.dtype)))
        elif alloc.kind == "ExternalOutput":
            out_names.append(name)
            out_avals.append(jax.core.ShapedArray(
                tuple(alloc.tensor_shape), mybir.dt.np(alloc.dtype)))
    n_params = len(in_names)
    names_all = list(in_names)
    if partition_name is not None:
        names_all.append(partition_name)

    def _body(*args):
        operands = list(args)
        if partition_name is not None:
            operands.append(partition_id_tensor())
        outs = _bass_exec_p.bind(
            *operands,
            out_avals=tuple(out_avals),
            in_names=tuple(names_all),
            out_names=tuple(out_names),
            lowering_input_output_aliases=(),
            sim_require_finite=True,
            sim_require_nnan=True,
            nc=nc,
        )
        return tuple(outs)

    devices = jax.devices()[:NCORES]
    assert len(devices) == NCORES
    mesh = Mesh(np.asarray(devices), ("core",))
    sharding = NamedSharding(mesh, PartitionSpec("core"))
    jitted = jax.jit(
        shard_map(_body, mesh=mesh,
                  in_specs=(PartitionSpec("core"),) * n_params,
                  out_specs=(PartitionSpec("core"),) * len(out_names),
                  check_rep=False),
        keep_unused=True)
    global_shapes = [jax.ShapeDtypeStruct((NCORES * s[0], *s[1:]), d)
                     for s, d in in_shapes]
    compiled = jitted.lower(*global_shapes).compile()
    return compiled, in_names, out_names, devices, sharding


def _get_state(T):
    if T not in _CACHE:
        nc = _build_program(T)
        _CACHE[T] = (nc, _make_runner(nc))
    return _CACHE[T]


def _prep_weights(W_ar, b_ar, W_ma, b_ma):
    # wtap: tap lag d in 1..8, tile (k, m):
    #   wtap[p, ((d-1)*4 + k*2 + m)*128 + q] = W_ar[m*128+q, (8-d)*256 + k*128 + p]
    wt = np.empty((128, ARSZ * 4 * 128), np.float32)
    for d in range(1, 9):
        blkc = W_ar[:, (8 - d) * F:(9 - d) * F]  # [fout, fin]
        for k in range(2):
            for m in range(2):
                tilev = blkc[m * 128:(m + 1) * 128, k * 128:(k + 1) * 128].T
                i = ((d - 1) * 4 + k * 2 + m) * 128
                wt[:, i:i + 128] = tilev
    # wma: tile (kk, m): wma[p, (kk*2+m)*128+q] = W_ma[m*128+q, kk*128+p]
    wm = np.empty((128, 18 * 2 * 128), np.float32)
    WmaT = W_ma.T  # [2304, 256]
    for kk in range(18):
        for m in range(2):
            i = (kk * 2 + m) * 128
            wm[:, i:i + 128] = WmaT[kk * 128:(kk + 1) * 128,
                                    m * 128:(m + 1) * 128]
    pack = np.zeros((128, WPACK_COLS), np.float32)
    pack[:, 0:4096] = wt
    pack[:, 4096:8704] = wm
    pack[0, 8704:8960] = b_ar
    pack[:, 8960] = b_ma[:128]
    pack[:, 8961] = b_ma[128:]
    return _bf16(pack)


def _unshard_core(yc, T):
    # y[b, t, m*128+p] = yc[p, t*16+m*8+b]  for one core
    a = yc.reshape(128, T, 2, BL).transpose(3, 1, 2, 0)
    return a.astype(np.float32).reshape(BL, T, F)


def kernel(x, W_ar, b_ar, W_ma, b_ma):
    import jax

    T = int(os.environ.get("ARMA_T", T_FULL))
    x = np.asarray(x, np.float32)[:, :T, :]
    W_ar = np.asarray(W_ar, np.float32)
    b_ar = np.asarray(b_ar, np.float32)
    W_ma = np.asarray(W_ma, np.float32)
    b_ma = np.asarray(b_ma, np.float32)

    nc, (compiled, in_names, out_names, devices, sharding) = _get_state(T)

    wdict = {"wpack": _prep_weights(W_ar, b_ar, W_ma, b_ma)}

    if int(os.environ.get("ARMA_TRACE", "0")):
        from concourse.bass_utils import run_bass_kernel_spmd
        in_maps = []
        for c in range(NCORES):
            m = dict(wdict)
            m["xt"] = x[c * BL:(c + 1) * BL].reshape(BL * T, F).astype(
                ml_dtypes.bfloat16)
            in_maps.append({n: m[n] for n in in_names})
        try:
            res = run_bass_kernel_spmd(nc, in_maps, list(range(NCORES)),
                                       trace=True)
        except ModuleNotFoundError:
            # NTFF profiling hooks unavailable in this container
            res = run_bass_kernel_spmd(nc, in_maps, list(range(NCORES)))
        kernel.last_results = res
        out = np.empty((B_TOT, T, F), np.float32)
        for c in range(NCORES):
            out[c * BL:(c + 1) * BL] = _unshard_core(
                np.asarray(res.results[c]["yout"]), T)
        return out

    kernel.last_results = None
    import time as _time
    _dbg = int(os.environ.get("ARMA_DEBUG", "0"))
    _t = _time.perf_counter

    def _mark(label, t0):
        if _dbg:
            print(f"[arma] {label}: {_t() - t0:.2f}s", flush=True)
        return _t()

    t0 = _t()
    # Per-core shards go to their device as soon as they're prepared, so
    # host-side bf16 casts overlap the (~40MB/s for real data) tunnel
    # transfers.
    x_shards = []
    for c in range(NCORES):
        xc = x[c * BL:(c + 1) * BL].reshape(BL * T, F).astype(
            ml_dtypes.bfloat16)
        x_shards.append(jax.device_put(xc, devices[c]))
    w_shards = {n: [jax.device_put(wdict[n], d) for d in devices]
                for n in in_names if n != "xt"}
    t0 = _mark("puts dispatched", t0)

    def garr(name, shards):
        shp = shards[0].shape
        return jax.make_array_from_single_device_arrays(
            (NCORES * shp[0], *shp[1:]), sharding, shards)

    args = []
    for n in in_names:
        args.append(garr(n, x_shards if n == "xt" else w_shards[n]))
    t0 = _mark("garr", t0)
    outs = compiled(*args)
    yg = outs[0]
    jax.block_until_ready(yg)
    t0 = _mark("exec+transfers", t0)
    out = np.empty((B_TOT, T, F), np.float32)
    shard_map_ = {s.index[0].start or 0: s.data for s in yg.addressable_shards}
    datas = [shard_map_[c * 128] for c in range(NCORES)]
    for d in datas:  # start all D2H copies before converting any
        try:
            d.copy_to_host_async()
        except Exception:
            pass
    for c in range(NCORES):
        out[c * BL:(c + 1) * BL] = _unshard_core(np.asarray(datas[c]), T)
    _mark("fetch+unshard", t0)
    return out


def _enable_jax_cache():
    try:
        import jax
        jax.config.update("jax_compilation_cache_dir",
                          "/var/tmp/arma_jax_cache")
        jax.config.update("jax_persistent_cache_min_compile_time_secs", 0.0)
        jax.config.update("jax_persistent_cache_min_entry_size_bytes", -1)
    except Exception:
        pass


# One-time setup (imports, program build, AOT compile) at module import.
if not int(os.environ.get("ARMA_NO_PREBUILD", "0")):
    try:
        _enable_jax_cache()
        _get_state(int(os.environ.get("ARMA_T", T_FULL)))
    except Exception:
        _CACHE.clear()
